# revision 1
# baseline (speedup 1.0000x reference)
"""KAN-attention Trainium2 kernel (8 NeuronCores, SPMD).

Math: for each batch b,
    q = x Wq^T + bq ; k = x Wk^T + bk ; v = x Wv^T + bv
    kq = q basis^T ; kk = k basis^T           (rank-16 projections)
    out = softmax(kq kk^T / 32) v

Folding (host): kq = x (basis Wq)^T + basis bq  == x Bq^T + cq, same for k.
So the 1024x1024 Q/K matmuls are never done. The softmax scale s=1/32 is
folded into Bq/cq. bv is folded out entirely: with unnormalized weights
e = exp(logits), out = (e @ v_nb)/rowsum + bv where v_nb = x Wv^T.

Sharding: core c = 2b+h handles batch b and key-half h (1024 of 2048 keys).
Each core computes p = e_half @ v_half (2048x1024) and r = rowsum_half
(2048). Host: out_b = (p0 + p1)/(r0 + r1) + bv. Key-halves are made
uniform across cores by rotating the sequence axis on the host (keys
always occupy positions 0:1024 of the shipped x^T), and un-rotating p/r.

Device layouts (per core): everything keeps the contraction dim on SBUF
partitions; logits are produced directly transposed (keys on partitions)
so the attention matmul needs no on-chip transpose; softmax normalization
is deferred to the host so no partition-axis reduction is needed beyond a
ones-column matmul that rides the same stationary weights.
"""

import os
import sys

sys.path.insert(0, "/opt/trn_rl_repo")

import numpy as np

DIM = 1024
SEQ = 2048
NF = 16
NCORES = 8

# matmul operand dtype: float32r streams 1 col/cycle (vs 4 for float32)
_DT_MM_NAME = os.environ.get("KAN_DT", "float32r")

_cache = {}


def _build(dt_mm_name):
    import concourse.bass as bass
    import concourse.tile as tile
    from concourse import bacc, mybir

    dt = mybir.dt
    DTM = getattr(dt, dt_mm_name)
    f32 = dt.float32

    nc = bacc.Bacc("TRN2", target_bir_lowering=False)

    xt = nc.declare_dram_parameter("xt", [DIM, SEQ], DTM, isOutput=False)
    wvt = nc.declare_dram_parameter("wvt", [DIM, DIM], DTM, isOutput=False)
    bqkt = nc.declare_dram_parameter("bqkt", [DIM, 256], DTM, isOutput=False)
    cqk = nc.declare_dram_parameter("cqk", [128, 2], f32, isOutput=False)
    ones_in = nc.declare_dram_parameter("ones", [128, 1], DTM, isOutput=False)
    p_out = nc.declare_dram_parameter("p", [SEQ, DIM], f32, isOutput=True)
    r_out = nc.declare_dram_parameter("r", [1, SEQ], f32, isOutput=True)

    xt_r = xt.rearrange("(o p) l -> p o l", p=128)    # (128, 8, 2048)
    wvt_r = wvt.rearrange("(o p) e -> p o e", p=128)  # (128, 8, 1024)
    bqkt_r = bqkt.rearrange("(o p) f -> p o f", p=128)

    MHALF = SEQ // 2  # keys this core owns (always cols 0:1024 of xt)

    with tile.TileContext(nc) as tc:
        with (
            tc.tile_pool(name="res", bufs=1) as res,
            tc.tile_pool(name="expp", bufs=6) as expp,
            tc.tile_pool(name="outp", bufs=3) as outp,
        ):
            xt_sb = res.tile([128, 8, SEQ], DTM)
            wvt_sb = res.tile([128, 8, DIM], DTM)
            bqkt_sb = res.tile([128, 8, 256], DTM)
            cqk_sb = res.tile([128, 2], f32)
            prime_sb = res.tile([128, 2], f32)
            kanq_sb = res.tile([128, SEQ], DTM)   # rows 0:16 data, 16:128 zero
            kank_sb = res.tile([128, MHALF], DTM)
            v_sb = res.tile([128, 8, DIM], DTM)   # keys on partitions
            ones_sb = res.tile([128, 1], DTM)
            r_sb = res.tile([1, SEQ], f32)

            # loads: key-half of xt and wvt first (v-stage inputs), small
            # tensors next, query half of xt last (streams in during v/kan)
            nc.sync.dma_start(out=bqkt_sb[:], in_=bqkt_r[:])
            nc.sync.dma_start(out=cqk_sb[:], in_=cqk[:])
            for lg in range(2):
                for dc in range(8):
                    nc.sync.dma_start(
                        out=xt_sb[:, dc, lg * 512:(lg + 1) * 512],
                        in_=xt_r[:, dc, lg * 512:(lg + 1) * 512],
                    )
            for dc in range(8):
                nc.sync.dma_start(out=wvt_sb[:, dc, :], in_=wvt_r[:, dc, :])
            for lg in range(2, 4):
                for dc in range(8):
                    nc.sync.dma_start(
                        out=xt_sb[:, dc, lg * 512:(lg + 1) * 512],
                        in_=xt_r[:, dc, lg * 512:(lg + 1) * 512],
                    )

            nc.sync.dma_start(out=ones_sb[:], in_=ones_in[:])
            # absorb the bias-DMA wait on the ACT engine so later bias
            # activations carry a single (PE) wait: AC struct has 1 slot
            nc.scalar.copy(out=prime_sb[:], in_=cqk_sb[:])

            # kan projections: (16, l) = Bqk^T.T @ xt, biased
            with tc.tile_pool(name="pskan", bufs=2, space="PSUM") as pskan:
                for lg in range(4):
                    ps = pskan.tile([128, 512], f32)
                    for dc in range(8):
                        nc.tensor.matmul(
                            ps,
                            bqkt_sb[:, dc, 0:128],
                            xt_sb[:, dc, lg * 512:(lg + 1) * 512],
                            start=(dc == 0),
                            stop=(dc == 7),
                        )
                    nc.scalar.activation(
                        out=kanq_sb[:, lg * 512:(lg + 1) * 512],
                        in_=ps,
                        func=mybir.ActivationFunctionType.Identity,
                        bias=cqk_sb[:, 0:1],
                        scale=1.0,
                    )
                for mg in range(2):
                    ps = pskan.tile([128, 512], f32)
                    for dc in range(8):
                        nc.tensor.matmul(
                            ps,
                            bqkt_sb[:, dc, 128:256],
                            xt_sb[:, dc, mg * 512:(mg + 1) * 512],
                            start=(dc == 0),
                            stop=(dc == 7),
                        )
                    nc.scalar.activation(
                        out=kank_sb[:, mg * 512:(mg + 1) * 512],
                        in_=ps,
                        func=mybir.ActivationFunctionType.Identity,
                        bias=cqk_sb[:, 1:2],
                        scale=1.0,
                    )

            # v (no bias): keys on partitions: v[m, e] over 8 m-chunks
            with tc.tile_pool(name="psv", bufs=4, space="PSUM") as psv:
                for mc in range(8):
                    for dg in range(2):
                        ps = psv.tile([128, 512], f32)
                        for dc in range(8):
                            nc.tensor.matmul(
                                ps,
                                xt_sb[:, dc, mc * 128:(mc + 1) * 128],
                                wvt_sb[:, dc, dg * 512:(dg + 1) * 512],
                                start=(dc == 0),
                                stop=(dc == 7),
                            )
                        nc.vector.tensor_copy(
                            out=v_sb[:, mc, dg * 512:(dg + 1) * 512], in_=ps
                        )

            # attention: logits^T (keys on partitions) -> exp -> @ v
            with (
                tc.tile_pool(name="pso", bufs=1, space="PSUM") as pso,
                tc.tile_pool(name="psl", bufs=2, space="PSUM") as psl,
                tc.tile_pool(name="psr", bufs=2, space="PSUM") as psr,
            ):
                for lg in range(8):  # query groups of 256
                    po = [
                        pso.tile([128, DIM], f32, name=f"po{i}")
                        for i in range(2)
                    ]
                    pr = psr.tile([1, 256], f32)
                    for mc in range(8):
                        pl = psl.tile([128, 256], f32)
                        nc.tensor.matmul(
                            pl,
                            kank_sb[:, mc * 128:(mc + 1) * 128],
                            kanq_sb[:, lg * 256:(lg + 1) * 256],
                            start=True,
                            stop=True,
                        )
                        et = expp.tile([128, 256], DTM)
                        nc.scalar.activation(
                            out=et, in_=pl, func=mybir.ActivationFunctionType.Exp
                        )
                        nc.tensor.matmul(
                            pr, ones_sb, et, start=(mc == 0), stop=(mc == 7)
                        )
                        for lc in range(2):
                            lhs = et[:, lc * 128:(lc + 1) * 128]
                            nc.tensor.matmul(
                                po[lc][:, 0:512], lhs, v_sb[:, mc, 0:512],
                                start=(mc == 0), stop=(mc == 7),
                            )
                            nc.tensor.matmul(
                                po[lc][:, 512:1024], lhs, v_sb[:, mc, 512:1024],
                                start=(mc == 0), stop=(mc == 7),
                            )
                    nc.vector.tensor_copy(
                        out=r_sb[:, lg * 256:(lg + 1) * 256], in_=pr
                    )
                    for lc in range(2):
                        ot = outp.tile([128, DIM], f32)
                        nc.vector.tensor_copy(out=ot[:, 0:512], in_=po[lc][:, 0:512])
                        nc.vector.tensor_copy(out=ot[:, 512:1024], in_=po[lc][:, 512:1024])
                        row0 = (lg * 256 + lc * 128)
                        nc.sync.dma_start(
                            out=p_out[row0:row0 + 128, :], in_=ot[:]
                        )
            nc.sync.dma_start(out=r_out[:], in_=r_sb[:])

    nc.compile()
    return nc


def _get_nc():
    if _DT_MM_NAME not in _cache:
        _cache[_DT_MM_NAME] = _build(_DT_MM_NAME)
    return _cache[_DT_MM_NAME]


def kernel(x, basis, Wq, bq, Wk, bk, Wv, bv, _trace=False):
    from concourse.bass_utils import run_bass_kernel_spmd

    x = np.asarray(x, dtype=np.float32)
    basis = np.asarray(basis, dtype=np.float32)
    Wq = np.asarray(Wq, dtype=np.float32)
    bq = np.asarray(bq, dtype=np.float32)
    Wk = np.asarray(Wk, dtype=np.float32)
    bk = np.asarray(bk, dtype=np.float32)
    Wv = np.asarray(Wv, dtype=np.float32)
    bv = np.asarray(bv, dtype=np.float32)

    # q = x @ Wq.T + bq ; kan_q = q @ basis.T = x @ (basis @ Wq).T + basis @ bq
    s = 1.0 / np.sqrt(np.float32(DIM))
    Bq = (basis @ Wq) * s            # (16, 1024), softmax scale folded into q side
    cq = (basis @ bq) * s
    Bk = basis @ Wk
    ck = basis @ bk
    bqkt_np = np.zeros((DIM, 256), dtype=np.float32)
    bqkt_np[:, 0:NF] = Bq.T
    bqkt_np[:, 128:128 + NF] = Bk.T
    cqk128 = np.zeros((128, 2), dtype=np.float32)
    cqk128[:NF, 0] = cq
    cqk128[:NF, 1] = ck
    wvt_np = np.ascontiguousarray(Wv.T, dtype=np.float32)  # v = x @ Wv.T -> rhs Wv.T (din, e)

    nc = _get_nc()
    in_maps = []
    for c in range(NCORES):
        b, h = c // 2, c % 2
        xtb = x[b].T  # (1024, 2048)
        if h == 0:
            xt2 = xtb
        else:
            xt2 = np.concatenate([xtb[:, 1024:], xtb[:, :1024]], axis=1)
        in_maps.append(
            {
                "xt": np.ascontiguousarray(xt2, dtype=np.float32),
                "wvt": wvt_np,
                "bqkt": bqkt_np,
                "cqk": cqk128,
                "ones": np.ones((128, 1), dtype=np.float32),
            }
        )

    res = run_bass_kernel_spmd(nc, in_maps, list(range(NCORES)), trace=_trace)
    kernel.last_results = res

    out = np.empty((4, SEQ, DIM), dtype=np.float32)
    for b in range(4):
        p0 = res.results[2 * b]["p"]
        p1 = res.results[2 * b + 1]["p"]
        r0 = res.results[2 * b]["r"][0]
        r1 = res.results[2 * b + 1]["r"][0]
        p1 = np.roll(p1, 1024, axis=0)
        r1 = np.roll(r1, 1024, axis=0)
        out[b] = (p0 + p1) / (r0 + r1)[:, None] + bv
    return out



# revision 5
# speedup vs baseline: 2.3094x; 2.3094x over previous
"""KAN-attention Trainium2 kernel (8 NeuronCores, SPMD), fp8 DoubleRow version.

Math per batch b:
    q = x Wq^T + bq ; k = x Wk^T + bk ; v = x Wv^T
    kq = q basis^T ; kk = k basis^T            (rank-16)
    out = softmax(kq kk^T / 32) v + bv

Folding: kq = x Bq^T + cq with Bq = basis Wq (host).  Writing e = exp(l)
= 1 + delta, the attention numerator splits as e@v = colsum(v) + delta@v
where colsum(v) is computed EXACTLY on the host (tiny matvec).  The
device only computes p = delta@v and r = rowsum(delta); fp8 quantization
error is then suppressed by |delta| ~ 0.04, so all heavy matmuls run in
fp8e4m3 with DoubleRow (2 contraction rows per PE cell -> 4x fewer PE
cycles than fp32).

Sharding: core c = 2b+h handles batch b and key-half h (1024 of 2048
keys), sequence rotated on host so keys sit at cols 0:1024 of xt.
Host combine: out_b = (p0+p1 + c0+c1) / (2048 + r0+r1) + bv.

Device dataflow (per core), everything fp8 except where noted:
  kan:    psq[16,2048]  = sum_g Bq8[128,2,16].T @ xt[128,2,512]   (DR)
          kanq[16,2048] (bf16) = psq + cq      (ACT, bias)
  v:      psv[128,512]  = sum_g xt[128,2,128k].T @ wvt[128,2,512] (DR)
          v8[128,(g,t),1024] (fp8)             (DVE copy)
  logits: psl[128k,512q] = kank[16,128k].T @ kanq[16,512q]  (bf16, K=16)
  exp:    e[128,512] f32 = Exp(psl * 2^-15)    (ACT)
  delta:  d8[128,(kc),2048] = e - 1 -> fp8     (DVE/Pool)
  attn:   pso[128q,1024e] += d8[128,2,128q].T @ v8[128,2,512e]    (DR)
  rowsum: psr[1,512] += ones[128,2,1].T @ d8[128,2,512q]          (DR)
  out:    p bf16 via engine copy + DMA; r f32.
"""

import os
import sys

sys.path.insert(0, "/opt/trn_rl_repo")

import math

import numpy as np

DIM = 1024
SEQ = 2048
NF = 16
NCORES = 8
MH = 1024  # keys per core

_cache = {}


def _build():
    import concourse.bass as bass
    import concourse.tile as tile
    from concourse import bacc, mybir

    dt = mybir.dt
    f8 = dt.float8e4
    bf16 = dt.bfloat16
    f32 = dt.float32
    DR = mybir.MatmulPerfMode.DoubleRow
    EXPS = 1.0 / 32768.0  # softmax scale 1/32 / (SB*SB) with SB=32

    nc = bacc.Bacc("TRN2", target_bir_lowering=False)

    xt = nc.declare_dram_parameter("xt", [DIM, SEQ], f8, isOutput=False)
    wvt = nc.declare_dram_parameter("wvt", [DIM, DIM], f8, isOutput=False)
    bqk = nc.declare_dram_parameter("bqk", [DIM, 32], f8, isOutput=False)
    cqk = nc.declare_dram_parameter("cqk", [NF, 2], f32, isOutput=False)
    p_out = nc.declare_dram_parameter("p", [SEQ, DIM], bf16, isOutput=True)
    r_out = nc.declare_dram_parameter("r", [1, SEQ], f32, isOutput=True)

    xt_r = xt.rearrange("(o p) l -> p o l", p=128)    # (128, 8, 2048), o=(g,t)
    wvt_r = wvt.rearrange("(o p) e -> p o e", p=128)  # (128, 8, 1024)
    bqk_r = bqk.rearrange("(o p) f -> p o f", p=128)  # (128, 8, 32)

    with tile.TileContext(nc) as tc:
        with tc.tile_pool(name="res", bufs=1) as res:
            xt_sb = res.tile([128, 8, SEQ], f8)
            wvt_sb = res.tile([128, 8, DIM], f8)
            bqk_sb = res.tile([128, 8, 32], f8)
            cqk_sb = res.tile([NF, 2], f32)
            # [128, 2, 16] so the DoubleRow ldweights k-slot stride (16 B)
            # satisfies the ISA step%16==0 constraint; only [:, :, 0:1] is used
            ones_sb = res.tile([128, 2, 16], f8)
            kanq_sb = res.tile([NF, SEQ], bf16)
            kank_sb = res.tile([NF, MH], bf16)
            v_sb = res.tile([128, 4, 2, DIM], f8)     # keys (g,t) on dims 1,2
            d_sb = res.tile([128, 8, SEQ], f8)        # delta^T, dim1 = key chunk
            r_sb = res.tile([1, SEQ], f32)

            nc.vector.memset(ones_sb, 1.0)

            # input DMAs: key-half of xt + wvt first so the v matmuls can
            # start early; query half streams in behind them
            nc.sync.dma_start(out=bqk_sb[:], in_=bqk_r[:])
            nc.sync.dma_start(out=cqk_sb[:], in_=cqk[:])
            nc.sync.dma_start(out=xt_sb[:, :, 0:MH], in_=xt_r[:, :, 0:MH])
            nc.sync.dma_start(out=wvt_sb[:], in_=wvt_r[:])
            nc.sync.dma_start(out=xt_sb[:, :, MH:SEQ], in_=xt_r[:, :, MH:SEQ])

            with (
                tc.tile_pool(name="pskan", bufs=1, space="PSUM") as pskan,
                tc.tile_pool(name="psv", bufs=2, space="PSUM") as psv,
            ):
                # kan projections (DoubleRow fp8): psq [16, 2048], psk [16, 1024]
                psq = pskan.tile([NF, SEQ], f32)
                psk = pskan.tile([NF, MH], f32)
                for qc in range(4):
                    for g in range(4):
                        nc.tensor.matmul(
                            psq[:, qc * 512:(qc + 1) * 512],
                            bqk_sb[:, 2 * g:2 * g + 2, 0:NF],
                            xt_sb[:, 2 * g:2 * g + 2, qc * 512:(qc + 1) * 512],
                            start=(g == 0), stop=(g == 3), perf_mode=DR,
                        )
                for qc in range(2):
                    for g in range(4):
                        nc.tensor.matmul(
                            psk[:, qc * 512:(qc + 1) * 512],
                            bqk_sb[:, 2 * g:2 * g + 2, NF:32],
                            xt_sb[:, 2 * g:2 * g + 2, qc * 512:(qc + 1) * 512],
                            start=(g == 0), stop=(g == 3), perf_mode=DR,
                        )
                nc.scalar.activation(
                    out=kanq_sb[:], in_=psq,
                    func=mybir.ActivationFunctionType.Identity,
                    bias=cqk_sb[:, 0:1], scale=1.0,
                )
                nc.scalar.activation(
                    out=kank_sb[:], in_=psk,
                    func=mybir.ActivationFunctionType.Identity,
                    bias=cqk_sb[:, 1:2], scale=1.0,
                )

                # v projection (DoubleRow fp8): v8 with keys on partitions
                for kc in range(8):
                    for eh in range(2):
                        ps = psv.tile([128, 512], f32)
                        for g in range(4):
                            nc.tensor.matmul(
                                ps,
                                xt_sb[:, 2 * g:2 * g + 2, kc * 128:(kc + 1) * 128],
                                wvt_sb[:, 2 * g:2 * g + 2, eh * 512:(eh + 1) * 512],
                                start=(g == 0), stop=(g == 3), perf_mode=DR,
                            )
                        nc.vector.tensor_copy(
                            out=v_sb[:, kc // 2, kc % 2, eh * 512:(eh + 1) * 512],
                            in_=ps,
                        )

            with (
                tc.tile_pool(name="psl", bufs=2, space="PSUM") as pslp,
                tc.tile_pool(name="pso", bufs=2, space="PSUM") as psop,
                tc.tile_pool(name="pss", bufs=2, space="PSUM") as pssp,
                tc.tile_pool(name="ep", bufs=4) as ep,
                tc.tile_pool(name="pp", bufs=3) as pp,
            ):
                # logits -> exp -> delta, by 512-wide query groups
                for qc in range(4):
                    qs = slice(qc * 512, (qc + 1) * 512)
                    for mc in range(8):
                        pl = pslp.tile([128, 512], f32)
                        nc.tensor.matmul(
                            pl,
                            kank_sb[:, mc * 128:(mc + 1) * 128],
                            kanq_sb[:, qs],
                            start=True, stop=True,
                        )
                        et = ep.tile([128, 512], f32)
                        nc.scalar.activation(
                            out=et, in_=pl,
                            func=mybir.ActivationFunctionType.Exp,
                            scale=EXPS,
                        )
                        nc.vector.tensor_scalar_sub(
                            out=d_sb[:, mc, qs], in0=et, scalar1=1.0,
                        )
                    psr = pssp.tile([1, 512], f32)
                    for g in range(4):
                        nc.tensor.matmul(
                            psr,
                            ones_sb[:, :, 0:1],
                            d_sb[:, 2 * g:2 * g + 2, qs],
                            start=(g == 0), stop=(g == 3), perf_mode=DR,
                        )
                    nc.scalar.copy(out=r_sb[:, qs], in_=psr)

                # attention: p = delta @ v, by 128-query tiles
                for qc in range(16):
                    po = psop.tile([128, DIM], f32)
                    for g in range(4):
                        for eh in range(2):
                            nc.tensor.matmul(
                                po[:, eh * 512:(eh + 1) * 512],
                                d_sb[:, 2 * g:2 * g + 2, qc * 128:(qc + 1) * 128],
                                v_sb[:, g, :, eh * 512:(eh + 1) * 512],
                                start=(g == 0), stop=(g == 3), perf_mode=DR,
                            )
                    pt = pp.tile([128, DIM], bf16)
                    nc.scalar.copy(out=pt[:], in_=po)
                    nc.sync.dma_start(
                        out=p_out[qc * 128:(qc + 1) * 128, :], in_=pt[:]
                    )
                nc.sync.dma_start(out=r_out[:], in_=r_sb[:])

    nc.compile()
    return nc


def _get_nc():
    if "nc" not in _cache:
        _cache["nc"] = _build()
    return _cache["nc"]


def kernel(x, basis, Wq, bq, Wk, bk, Wv, bv, _trace=False):
    import ml_dtypes
    from concourse.bass_utils import run_bass_kernel_spmd

    f8 = ml_dtypes.float8_e4m3

    x = np.asarray(x, dtype=np.float32)
    basis = np.asarray(basis, dtype=np.float32)
    Wq = np.asarray(Wq, dtype=np.float32)
    bq = np.asarray(bq, dtype=np.float32)
    Wk = np.asarray(Wk, dtype=np.float32)
    bk = np.asarray(bk, dtype=np.float32)
    Wv = np.asarray(Wv, dtype=np.float32)
    bv = np.asarray(bv, dtype=np.float32)

    SB = np.float32(32.0)
    Bq = (basis @ Wq) * SB            # (16, 1024); exp scale 2^-15 on device
    Bk = (basis @ Wk) * SB
    cq = (basis @ bq) * SB
    ck = (basis @ bk) * SB
    bqk_np = np.zeros((DIM, 32), dtype=np.float32)
    bqk_np[:, 0:NF] = Bq.T
    bqk_np[:, NF:32] = Bk.T
    bqk_np = bqk_np.astype(f8)
    cqk_np = np.stack([cq, ck], axis=1).astype(np.float32)  # (16, 2)
    wvt_np = np.ascontiguousarray(Wv.T).astype(f8)          # (din, e)

    nc = _get_nc()
    in_maps = []
    for c in range(NCORES):
        b, h = c // 2, c % 2
        xtb = x[b].T  # (1024, 2048)
        if h == 1:
            xtb = np.concatenate([xtb[:, MH:], xtb[:, :MH]], axis=1)
        in_maps.append(
            {
                "xt": np.ascontiguousarray(xtb).astype(f8),
                "wvt": wvt_np,
                "bqk": bqk_np,
                "cqk": cqk_np,
            }
        )

    res = run_bass_kernel_spmd(nc, in_maps, list(range(NCORES)), trace=_trace)
    kernel.last_results = res

    # exact colsum-of-v correction on host: c_half = (sum over keys of x) @ Wv.T
    out = np.empty((4, SEQ, DIM), dtype=np.float32)
    for b in range(4):
        c0 = (x[b, :MH, :].sum(axis=0, dtype=np.float64) @ Wv.T.astype(np.float64))
        c1 = (x[b, MH:, :].sum(axis=0, dtype=np.float64) @ Wv.T.astype(np.float64))
        p0 = res.results[2 * b]["p"].astype(np.float32)
        p1 = res.results[2 * b + 1]["p"].astype(np.float32)
        r0 = res.results[2 * b]["r"][0]
        r1 = res.results[2 * b + 1]["r"][0]
        p1 = np.roll(p1, MH, axis=0)
        r1 = np.roll(r1, MH, axis=0)
        num = p0 + p1 + (c0 + c1).astype(np.float32)[None, :]
        den = np.float32(SEQ) + r0 + r1
        out[b] = num / den[:, None] + bv
    return out


# revision 9
# speedup vs baseline: 2.3844x; 1.0325x over previous
"""KAN-attention Trainium2 kernel (8 NeuronCores, SPMD), fp8 DoubleRow version.

Math per batch b:
    q = x Wq^T + bq ; k = x Wk^T + bk ; v = x Wv^T
    kq = q basis^T ; kk = k basis^T            (rank-16)
    out = softmax(kq kk^T / 32) v + bv

Folding: kq = x Bq^T + cq with Bq = basis Wq (host).  Writing e = exp(l)
= 1 + delta, the attention numerator splits as e@v = colsum(v) + delta@v
where colsum(v) is computed EXACTLY on the host (tiny matvec).  The
device only computes p = delta@v and r = rowsum(delta); fp8 quantization
error is then suppressed by |delta| ~ 0.04, so all heavy matmuls run in
fp8e4m3 with DoubleRow (2 contraction rows per PE cell -> 4x fewer PE
cycles than fp32).

Sharding: core c = 2b+h handles batch b and key-half h (1024 of 2048
keys), sequence rotated on host so keys sit at cols 0:1024 of xt.
Host combine: out_b = (p0+p1 + c0+c1) / (2048 + r0+r1) + bv.

Device dataflow (per core), everything fp8 except where noted:
  kan:    psq[16,2048]  = sum_g Bq8[128,2,16].T @ xt[128,2,512]   (DR)
          kanq[16,2048] (bf16) = psq + cq      (ACT, bias)
  v:      psv[128,512]  = sum_g xt[128,2,128k].T @ wvt[128,2,512] (DR)
          v8[128,(g,t),1024] (fp8)             (DVE copy)
  logits: psl[128k,512q] = kank[16,128k].T @ kanq[16,512q]  (bf16, K=16)
  exp:    e[128,512] f32 = Exp(psl * 2^-15)    (ACT)
  delta:  d8[128,(kc),2048] = e - 1 -> fp8     (DVE/Pool)
  attn:   pso[128q,1024e] += d8[128,2,128q].T @ v8[128,2,512e]    (DR)
  rowsum: psr[1,512] += ones[128,2,1].T @ d8[128,2,512q]          (DR)
  out:    p bf16 via engine copy + DMA; r f32.
"""

import os
import sys

sys.path.insert(0, "/opt/trn_rl_repo")

import math

import numpy as np

DIM = 1024
SEQ = 2048
NF = 16
NCORES = 8
MH = 1024  # keys per core

_cache = {}


def _build():
    import concourse.bass as bass
    import concourse.tile as tile
    from concourse import bacc, mybir

    dt = mybir.dt
    f8 = dt.float8e4
    bf16 = dt.bfloat16
    f32 = dt.float32
    DR = mybir.MatmulPerfMode.DoubleRow
    EXPS = 1.0 / 32768.0  # softmax scale 1/32 / (SB*SB) with SB=32

    nc = bacc.Bacc("TRN2", target_bir_lowering=False)

    xt = nc.declare_dram_parameter("xt", [DIM, SEQ], f8, isOutput=False)
    wvt = nc.declare_dram_parameter("wvt", [DIM, DIM], f8, isOutput=False)
    bqk = nc.declare_dram_parameter("bqk", [DIM, 32], f8, isOutput=False)
    cqk = nc.declare_dram_parameter("cqk", [NF, 2], f32, isOutput=False)
    p_out = nc.declare_dram_parameter("p", [SEQ, DIM], bf16, isOutput=True)
    r_out = nc.declare_dram_parameter("r", [1, SEQ], f32, isOutput=True)

    xt_r = xt.rearrange("(o p) l -> p o l", p=128)    # (128, 8, 2048), o=(g,t)
    wvt_r = wvt.rearrange("(o p) e -> p o e", p=128)  # (128, 8, 1024)
    bqk_r = bqk.rearrange("(o p) f -> p o f", p=128)  # (128, 8, 32)

    with tile.TileContext(nc) as tc:
        with tc.tile_pool(name="res", bufs=1) as res:
            xt_sb = res.tile([128, 8, SEQ], f8)
            wvt_sb = res.tile([128, 8, DIM], f8)
            bqk_sb = res.tile([128, 8, 32], f8)
            cqk_sb = res.tile([NF, 2], f32)
            # [128, 2, 16] so the DoubleRow ldweights k-slot stride (16 B)
            # satisfies the ISA step%16==0 constraint; only [:, :, 0:1] is used
            ones_sb = res.tile([128, 2, 16], f8)
            kanq_sb = res.tile([NF, SEQ], bf16)
            kank_sb = res.tile([NF, MH], bf16)
            v_sb = res.tile([128, 4, 2, DIM], f8)     # keys (g,t) on dims 1,2
            d_sb = res.tile([128, 8, SEQ], f8)        # delta^T, dim1 = key chunk
            r_sb = res.tile([1, SEQ], f32)

            nc.vector.memset(ones_sb, 1.0)

            # input DMAs: key-half of xt + wvt first so the v matmuls can
            # start early; query half streams in behind them
            nc.sync.dma_start(out=bqk_sb[:], in_=bqk_r[:])
            nc.sync.dma_start(out=cqk_sb[:], in_=cqk[:])
            nc.sync.dma_start(out=xt_sb[:, :, 0:MH], in_=xt_r[:, :, 0:MH])
            nc.sync.dma_start(out=wvt_sb[:], in_=wvt_r[:])
            nc.sync.dma_start(out=xt_sb[:, :, MH:SEQ], in_=xt_r[:, :, MH:SEQ])

            with (
                tc.tile_pool(name="pskan", bufs=2, space="PSUM") as pskan,
                tc.tile_pool(name="psv", bufs=2, space="PSUM") as psv,
            ):
                # kan projections (DoubleRow fp8), per 512-wide column group
                for qc in range(4):
                    psq = pskan.tile([NF, 512], f32, name="pskan_t")
                    for g in range(4):
                        nc.tensor.matmul(
                            psq,
                            bqk_sb[:, 2 * g:2 * g + 2, 0:NF],
                            xt_sb[:, 2 * g:2 * g + 2, qc * 512:(qc + 1) * 512],
                            start=(g == 0), stop=(g == 3), perf_mode=DR,
                        )
                    nc.scalar.activation(
                        out=kanq_sb[:, qc * 512:(qc + 1) * 512], in_=psq,
                        func=mybir.ActivationFunctionType.Identity,
                        bias=cqk_sb[:, 0:1], scale=1.0,
                    )
                for qc in range(2):
                    psk = pskan.tile([NF, 512], f32, name="pskan_t")
                    for g in range(4):
                        nc.tensor.matmul(
                            psk,
                            bqk_sb[:, 2 * g:2 * g + 2, NF:32],
                            xt_sb[:, 2 * g:2 * g + 2, qc * 512:(qc + 1) * 512],
                            start=(g == 0), stop=(g == 3), perf_mode=DR,
                        )
                    nc.scalar.activation(
                        out=kank_sb[:, qc * 512:(qc + 1) * 512], in_=psk,
                        func=mybir.ActivationFunctionType.Identity,
                        bias=cqk_sb[:, 1:2], scale=1.0,
                    )

                # v projection (DoubleRow fp8): v8 with keys on partitions
                for kc in range(8):
                    ps = psv.tile([128, DIM], f32)
                    for eh in range(2):
                        for g in range(4):
                            nc.tensor.matmul(
                                ps[:, eh * 512:(eh + 1) * 512],
                                xt_sb[:, 2 * g:2 * g + 2, kc * 128:(kc + 1) * 128],
                                wvt_sb[:, 2 * g:2 * g + 2, eh * 512:(eh + 1) * 512],
                                start=(g == 0), stop=(g == 3), perf_mode=DR,
                            )
                    nc.vector.tensor_copy(
                        out=v_sb[:, kc // 2, kc % 2, :], in_=ps,
                    )

            with (
                tc.tile_pool(name="psl", bufs=2, space="PSUM") as pslp,
                tc.tile_pool(name="pso", bufs=2, space="PSUM") as psop,
                tc.tile_pool(name="pss", bufs=2, space="PSUM") as pssp,
                tc.tile_pool(name="ep", bufs=4) as ep,
                tc.tile_pool(name="pp", bufs=3) as pp,
            ):
                # logits -> exp -> delta, by 512-wide query groups
                for qc in range(4):
                    qs = slice(qc * 512, (qc + 1) * 512)
                    for mc in range(8):
                        pl = pslp.tile([128, 512], f32)
                        nc.tensor.matmul(
                            pl,
                            kank_sb[:, mc * 128:(mc + 1) * 128],
                            kanq_sb[:, qs],
                            start=True, stop=True,
                        )
                        et = ep.tile([128, 512], f32)
                        nc.scalar.activation(
                            out=et, in_=pl,
                            func=mybir.ActivationFunctionType.Exp,
                            scale=EXPS,
                        )
                        nc.gpsimd.tensor_scalar_sub(
                            out=d_sb[:, mc, qs], in0=et, scalar1=1.0,
                        )
                    psr = pssp.tile([1, 512], f32)
                    for g in range(4):
                        nc.tensor.matmul(
                            psr,
                            ones_sb[:, :, 0:1],
                            d_sb[:, 2 * g:2 * g + 2, qs],
                            start=(g == 0), stop=(g == 3), perf_mode=DR,
                        )
                    nc.scalar.copy(out=r_sb[:, qs], in_=psr)

                # attention: p = delta @ v, by 128-query tiles
                for qc in range(16):
                    po = psop.tile([128, DIM], f32)
                    for g in range(4):
                        for eh in range(2):
                            nc.tensor.matmul(
                                po[:, eh * 512:(eh + 1) * 512],
                                d_sb[:, 2 * g:2 * g + 2, qc * 128:(qc + 1) * 128],
                                v_sb[:, g, :, eh * 512:(eh + 1) * 512],
                                start=(g == 0), stop=(g == 3), perf_mode=DR,
                            )
                    pt = pp.tile([128, DIM], bf16)
                    if qc % 2 == 0:
                        nc.vector.tensor_copy(out=pt[:], in_=po)
                    else:
                        nc.scalar.copy(out=pt[:], in_=po)
                    nc.sync.dma_start(
                        out=p_out[qc * 128:(qc + 1) * 128, :], in_=pt[:]
                    )
                nc.sync.dma_start(out=r_out[:], in_=r_sb[:])

    nc.compile()
    return nc


def _get_nc():
    if "nc" not in _cache:
        _cache["nc"] = _build()
    return _cache["nc"]


def kernel(x, basis, Wq, bq, Wk, bk, Wv, bv, _trace=False):
    import ml_dtypes
    from concourse.bass_utils import run_bass_kernel_spmd

    f8 = ml_dtypes.float8_e4m3

    x = np.asarray(x, dtype=np.float32)
    basis = np.asarray(basis, dtype=np.float32)
    Wq = np.asarray(Wq, dtype=np.float32)
    bq = np.asarray(bq, dtype=np.float32)
    Wk = np.asarray(Wk, dtype=np.float32)
    bk = np.asarray(bk, dtype=np.float32)
    Wv = np.asarray(Wv, dtype=np.float32)
    bv = np.asarray(bv, dtype=np.float32)

    SB = np.float32(32.0)
    Bq = (basis @ Wq) * SB            # (16, 1024); exp scale 2^-15 on device
    Bk = (basis @ Wk) * SB
    cq = (basis @ bq) * SB
    ck = (basis @ bk) * SB
    bqk_np = np.zeros((DIM, 32), dtype=np.float32)
    bqk_np[:, 0:NF] = Bq.T
    bqk_np[:, NF:32] = Bk.T
    bqk_np = bqk_np.astype(f8)
    cqk_np = np.stack([cq, ck], axis=1).astype(np.float32)  # (16, 2)
    wvt_np = np.ascontiguousarray(Wv.T).astype(f8)          # (din, e)

    nc = _get_nc()
    in_maps = []
    for c in range(NCORES):
        b, h = c // 2, c % 2
        xtb = x[b].T  # (1024, 2048)
        if h == 1:
            xtb = np.concatenate([xtb[:, MH:], xtb[:, :MH]], axis=1)
        in_maps.append(
            {
                "xt": np.ascontiguousarray(xtb).astype(f8),
                "wvt": wvt_np,
                "bqk": bqk_np,
                "cqk": cqk_np,
            }
        )

    res = run_bass_kernel_spmd(nc, in_maps, list(range(NCORES)), trace=_trace)
    kernel.last_results = res

    # exact colsum-of-v correction on host: c_half = (sum over keys of x) @ Wv.T
    out = np.empty((4, SEQ, DIM), dtype=np.float32)
    for b in range(4):
        c0 = (x[b, :MH, :].sum(axis=0, dtype=np.float64) @ Wv.T.astype(np.float64))
        c1 = (x[b, MH:, :].sum(axis=0, dtype=np.float64) @ Wv.T.astype(np.float64))
        p0 = res.results[2 * b]["p"].astype(np.float32)
        p1 = res.results[2 * b + 1]["p"].astype(np.float32)
        r0 = res.results[2 * b]["r"][0]
        r1 = res.results[2 * b + 1]["r"][0]
        p1 = np.roll(p1, MH, axis=0)
        r1 = np.roll(r1, MH, axis=0)
        num = p0 + p1 + (c0 + c1).astype(np.float32)[None, :]
        den = np.float32(SEQ) + r0 + r1
        out[b] = num / den[:, None] + bv
    return out


# revision 11
# speedup vs baseline: 3.3921x; 1.4226x over previous
"""KAN-attention Trainium2 kernel (8 NeuronCores, SPMD), fp8 DoubleRow version.

Math per batch b:
    q = x Wq^T + bq ; k = x Wk^T + bk ; v = x Wv^T
    kq = q basis^T ; kk = k basis^T            (rank-16)
    out = softmax(kq kk^T / 32) v + bv

Folding: kq = x Bq^T + cq with Bq = basis Wq (host).  Writing e = exp(l)
= 1 + delta, the attention numerator splits as e@v = colsum(v) + delta@v
where colsum(v) is computed EXACTLY on the host (tiny matvec).  The
device only computes p = delta@v and r = rowsum(delta); fp8 quantization
error is then suppressed by |delta| ~ 0.04, so all heavy matmuls run in
fp8e4m3 with DoubleRow (2 contraction rows per PE cell -> 4x fewer PE
cycles than fp32).

Sharding: core c = 2b+h handles batch b and key-half h (1024 of 2048
keys), sequence rotated on host so keys sit at cols 0:1024 of xt.
Host combine: out_b = (p0+p1 + c0+c1) / (2048 + r0+r1) + bv.

Device dataflow (per core), everything fp8 except where noted:
  kan:    psq[16,2048]  = sum_g Bq8[128,2,16].T @ xt[128,2,512]   (DR)
          kanq[16,2048] (bf16) = psq + cq      (ACT, bias)
  v:      psv[128,512]  = sum_g xt[128,2,128k].T @ wvt[128,2,512] (DR)
          v8[128,(g,t),1024] (fp8)             (DVE copy)
  logits: psl[128k,512q] = kank[16,128k].T @ kanq[16,512q]  (bf16, K=16)
  exp:    e[128,512] f32 = Exp(psl * 2^-15)    (ACT)
  delta:  d8[128,(kc),2048] = e - 1 -> fp8     (DVE/Pool)
  attn:   pso[128q,1024e] += d8[128,2,128q].T @ v8[128,2,512e]    (DR)
  rowsum: psr[1,512] += ones[128,2,1].T @ d8[128,2,512q]          (DR)
  out:    p bf16 via engine copy + DMA; r f32.
"""

import os
import sys

sys.path.insert(0, "/opt/trn_rl_repo")

import math

import numpy as np

DIM = 1024
SEQ = 2048
NF = 16
NCORES = 8
MH = 1024  # keys per core

_cache = {}


def _build():
    import concourse.bass as bass
    import concourse.tile as tile
    from concourse import bacc, mybir

    dt = mybir.dt
    f8 = dt.float8e4
    bf16 = dt.bfloat16
    f32 = dt.float32
    DR = mybir.MatmulPerfMode.DoubleRow
    EXPS = 1.0 / 32768.0  # softmax scale 1/32 / (SB*SB) with SB=32

    nc = bacc.Bacc("TRN2", target_bir_lowering=False)

    xt = nc.declare_dram_parameter("xt", [DIM, SEQ], f8, isOutput=False)
    wvt = nc.declare_dram_parameter("wvt", [DIM, DIM], f8, isOutput=False)
    bqk = nc.declare_dram_parameter("bqk", [DIM, 32], f8, isOutput=False)
    cqk = nc.declare_dram_parameter("cqk", [NF, 2], f32, isOutput=False)
    p_out = nc.declare_dram_parameter("p", [SEQ, DIM], bf16, isOutput=True)
    r_out = nc.declare_dram_parameter("r", [1, SEQ], f32, isOutput=True)

    xt_r = xt.rearrange("(o p) l -> p o l", p=128)    # (128, 8, 2048), o=(g,t)
    wvt_r = wvt.rearrange("(o p) e -> p o e", p=128)  # (128, 8, 1024)
    bqk_r = bqk.rearrange("(o p) f -> p o f", p=128)  # (128, 8, 32)

    with tile.TileContext(nc) as tc:
        with tc.tile_pool(name="res", bufs=1) as res:
            xt_sb = res.tile([128, 8, SEQ], f8)
            wvt_sb = res.tile([128, 8, DIM], f8)
            bqk_sb = res.tile([128, 8, 32], f8)
            cqk_sb = res.tile([NF, 2], f32)
            # [128, 2, 16] so the DoubleRow ldweights k-slot stride (16 B)
            # satisfies the ISA step%16==0 constraint; only [:, :, 0:1] is used
            ones_sb = res.tile([128, 2, 16], f8)
            kanq_sb = res.tile([NF, SEQ], bf16)
            kank_sb = res.tile([NF, MH], bf16)
            v_sb = res.tile([128, 4, 2, DIM], f8)     # keys (g,t) on dims 1,2
            d_sb = res.tile([128, 8, SEQ], f8)        # delta^T, dim1 = key chunk
            r_sb = res.tile([1, SEQ], f32)

            nc.vector.memset(ones_sb, 1.0)

            # input DMAs: key-half of xt + wvt first so the v matmuls can
            # start early; query half streams in behind them
            nc.sync.dma_start(out=bqk_sb[:], in_=bqk_r[:])
            nc.sync.dma_start(out=cqk_sb[:], in_=cqk[:])
            nc.sync.dma_start(out=xt_sb[:, :, 0:512], in_=xt_r[:, :, 0:512])
            nc.sync.dma_start(out=xt_sb[:, :, 512:MH], in_=xt_r[:, :, 512:MH])
            nc.sync.dma_start(out=wvt_sb[:], in_=wvt_r[:])
            nc.sync.dma_start(out=xt_sb[:, :, MH:SEQ], in_=xt_r[:, :, MH:SEQ])

            with (
                tc.tile_pool(name="psl", bufs=2, space="PSUM") as pslp,
                tc.tile_pool(name="ep", bufs=6) as ep,
                tc.tile_pool(name="pp", bufs=4) as pp,
            ):
                ncopy = {"i": 0}

                def kan_group(dst, col0, bias, tag):
                    ps = pskan.tile([NF, 512], f32, name="pskan_t")
                    for g in range(4):
                        nc.tensor.matmul(
                            ps,
                            bqk_sb[:, 2 * g:2 * g + 2, tag],
                            xt_sb[:, 2 * g:2 * g + 2, col0:col0 + 512],
                            start=(g == 0), stop=(g == 3), perf_mode=DR,
                        )
                    nc.scalar.activation(
                        out=dst[:, col0:col0 + 512], in_=ps,
                        func=mybir.ActivationFunctionType.Identity,
                        bias=bias, scale=1.0,
                    )

                def logits_group(qc):
                    qs = slice(qc * 512, (qc + 1) * 512)
                    for mc in range(8):
                        pl = pslp.tile([128, 512], f32, name="psl_t")
                        nc.tensor.matmul(
                            pl,
                            kank_sb[:, mc * 128:(mc + 1) * 128],
                            kanq_sb[:, qs],
                            start=True, stop=True,
                        )
                        et = ep.tile([128, 512], f32, name="ep_t")
                        nc.scalar.activation(
                            out=et, in_=pl,
                            func=mybir.ActivationFunctionType.Exp,
                            scale=EXPS,
                        )
                        i = qc * 8 + mc
                        eng = nc.vector if i % 3 == 2 else nc.gpsimd
                        eng.tensor_scalar_sub(
                            out=d_sb[:, mc, qs], in0=et, scalar1=1.0,
                        )

                def attn_group(g4):
                    # rowsum for this 512-query group
                    qs = slice(g4 * 512, (g4 + 1) * 512)
                    psr = pssp.tile([1, 512], f32, name="pss_t")
                    for g in range(4):
                        nc.tensor.matmul(
                            psr,
                            ones_sb[:, :, 0:1],
                            d_sb[:, 2 * g:2 * g + 2, qs],
                            start=(g == 0), stop=(g == 3), perf_mode=DR,
                        )
                    nc.vector.tensor_copy(out=r_sb[:, qs], in_=psr)
                    for qc in range(4 * g4, 4 * g4 + 4):
                        po = psop.tile([128, DIM], f32, name="pso_t")
                        for g in range(4):
                            for eh in range(2):
                                nc.tensor.matmul(
                                    po[:, eh * 512:(eh + 1) * 512],
                                    d_sb[:, 2 * g:2 * g + 2, qc * 128:(qc + 1) * 128],
                                    v_sb[:, g, :, eh * 512:(eh + 1) * 512],
                                    start=(g == 0), stop=(g == 3), perf_mode=DR,
                                )
                        pt = pp.tile([128, DIM], bf16, name="pp_t")
                        i = ncopy["i"]
                        ncopy["i"] += 1
                        if i % 4 == 3:
                            nc.scalar.copy(out=pt[:], in_=po)
                        else:
                            nc.vector.tensor_copy(out=pt[:], in_=po)
                        nc.sync.dma_start(
                            out=p_out[qc * 128:(qc + 1) * 128, :], in_=pt[:]
                        )

                with tc.tile_pool(name="pskan", bufs=2, space="PSUM") as pskan:
                    # kan over the key-half columns (queries 0:1024 + all keys)
                    kan_group(kanq_sb, 0, cqk_sb[:, 0:1], slice(0, NF))
                    kan_group(kank_sb, 0, cqk_sb[:, 1:2], slice(NF, 32))
                    kan_group(kank_sb, 512, cqk_sb[:, 1:2], slice(NF, 32))
                    kan_group(kanq_sb, 512, cqk_sb[:, 0:1], slice(0, NF))
                    logits_group(0)

                    with tc.tile_pool(name="psv", bufs=2, space="PSUM") as psv:
                        # v projection (DoubleRow fp8): keys on partitions
                        for kc in range(8):
                            ps = psv.tile([128, DIM], f32, name="psv_t")
                            for eh in range(2):
                                for g in range(4):
                                    nc.tensor.matmul(
                                        ps[:, eh * 512:(eh + 1) * 512],
                                        xt_sb[:, 2 * g:2 * g + 2, kc * 128:(kc + 1) * 128],
                                        wvt_sb[:, 2 * g:2 * g + 2, eh * 512:(eh + 1) * 512],
                                        start=(g == 0), stop=(g == 3), perf_mode=DR,
                                    )
                            if kc % 4 == 3:
                                nc.scalar.copy(out=v_sb[:, kc // 2, kc % 2, :], in_=ps)
                            else:
                                nc.vector.tensor_copy(
                                    out=v_sb[:, kc // 2, kc % 2, :], in_=ps
                                )
                        logits_group(1)

                    # query-half columns of xt have arrived during v
                    kan_group(kanq_sb, 1024, cqk_sb[:, 0:1], slice(0, NF))
                    kan_group(kanq_sb, 1536, cqk_sb[:, 0:1], slice(0, NF))

                with (
                    tc.tile_pool(name="pso", bufs=2, space="PSUM") as psop,
                    tc.tile_pool(name="pss", bufs=2, space="PSUM") as pssp,
                ):
                    attn_group(0)
                    logits_group(2)
                    attn_group(1)
                    logits_group(3)
                    attn_group(2)
                    attn_group(3)
                    nc.sync.dma_start(out=r_out[:], in_=r_sb[:])

    nc.compile()
    return nc


def _get_nc():
    if "nc" not in _cache:
        _cache["nc"] = _build()
    return _cache["nc"]


def kernel(x, basis, Wq, bq, Wk, bk, Wv, bv, _trace=False):
    import ml_dtypes
    from concourse.bass_utils import run_bass_kernel_spmd

    f8 = ml_dtypes.float8_e4m3

    x = np.asarray(x, dtype=np.float32)
    basis = np.asarray(basis, dtype=np.float32)
    Wq = np.asarray(Wq, dtype=np.float32)
    bq = np.asarray(bq, dtype=np.float32)
    Wk = np.asarray(Wk, dtype=np.float32)
    bk = np.asarray(bk, dtype=np.float32)
    Wv = np.asarray(Wv, dtype=np.float32)
    bv = np.asarray(bv, dtype=np.float32)

    SB = np.float32(32.0)
    Bq = (basis @ Wq) * SB            # (16, 1024); exp scale 2^-15 on device
    Bk = (basis @ Wk) * SB
    cq = (basis @ bq) * SB
    ck = (basis @ bk) * SB
    bqk_np = np.zeros((DIM, 32), dtype=np.float32)
    bqk_np[:, 0:NF] = Bq.T
    bqk_np[:, NF:32] = Bk.T
    bqk_np = bqk_np.astype(f8)
    cqk_np = np.stack([cq, ck], axis=1).astype(np.float32)  # (16, 2)
    wvt_np = np.ascontiguousarray(Wv.T).astype(f8)          # (din, e)

    nc = _get_nc()
    in_maps = []
    for c in range(NCORES):
        b, h = c // 2, c % 2
        xtb = x[b].T  # (1024, 2048)
        if h == 1:
            xtb = np.concatenate([xtb[:, MH:], xtb[:, :MH]], axis=1)
        in_maps.append(
            {
                "xt": np.ascontiguousarray(xtb).astype(f8),
                "wvt": wvt_np,
                "bqk": bqk_np,
                "cqk": cqk_np,
            }
        )

    res = run_bass_kernel_spmd(nc, in_maps, list(range(NCORES)), trace=_trace)
    kernel.last_results = res

    # exact colsum-of-v correction on host: c_half = (sum over keys of x) @ Wv.T
    out = np.empty((4, SEQ, DIM), dtype=np.float32)
    for b in range(4):
        c0 = (x[b, :MH, :].sum(axis=0, dtype=np.float64) @ Wv.T.astype(np.float64))
        c1 = (x[b, MH:, :].sum(axis=0, dtype=np.float64) @ Wv.T.astype(np.float64))
        p0 = res.results[2 * b]["p"].astype(np.float32)
        p1 = res.results[2 * b + 1]["p"].astype(np.float32)
        r0 = res.results[2 * b]["r"][0]
        r1 = res.results[2 * b + 1]["r"][0]
        p1 = np.roll(p1, MH, axis=0)
        r1 = np.roll(r1, MH, axis=0)
        num = p0 + p1 + (c0 + c1).astype(np.float32)[None, :]
        den = np.float32(SEQ) + r0 + r1
        out[b] = num / den[:, None] + bv
    return out
